# revision 5
# baseline (speedup 1.0000x reference)
"""Trainium2 Bass kernel for nn_DiffPoolPrompt (GCN conv + softmax pooling prompt).

Reference computation:
    h = x + sum(cluster_emb, 0)
    logits = GCNConv(h, W, bias, edge_index)   # sym-normalized, self-loops
    s = softmax(logits, axis=1)
    out = x + s @ cluster_emb

Distribution strategy (8 NeuronCores), v2:
  - Nodes sharded contiguously: core c owns nodes [c*12500, (c+1)*12500).
  - Canonical per-core layout: nodes sorted by in-degree into 12544 = 128x98
    cells (p = rank%128, r = rank//128); 4 "quarter" windows split the rows
    at QB (asymmetric: small first quarter so gathers start early, small last
    so the tail starts early). The per-window g-tables are published
    quarter-by-quarter: compact [rows,10] bf16 AllGather (~0.5MB out) as soon
    as phase B finishes that quarter's rows, then a DRAM->DRAM expand to the
    256B-strided layout dma_gather needs.
  - Edges are partitioned by destination core and by window (= source node's
    quarter). Each window uses its OWN destination layout, sorted by that
    window's per-node edge count, which drops slot padding from 2.05x to
    ~1.02x (846k -> ~422k gather tokens per core). The 4 per-window partial
    tables are re-aligned to the canonical layout with one extra gather pass
    (2 x 25088 tokens) before softmax.
  - x is uploaded channel-major in bf16 so phase B needs no transposes:
    g = dinv * (x @ W + cW) is 3 matmuls per 128-node chunk straight out of
    SBUF. The final s @ cluster_emb runs transposed as well (out^T = emb^T s^T)
    and the output ships bf16 channel-major; the host casts/transposes back.
  - Host work is index-only: binning, sorting, padding, gather index tables,
    and layout (un)permutation of rows.
"""

import os
import time
import numpy as np
import ml_dtypes

import concourse.bass as bass
import concourse.bacc as bacc
import concourse.tile as tile
import concourse.mybir as mybir
from concourse.bass_utils import run_bass_kernel_spmd
from concourse.masks import make_identity
from concourse import ap_utils

N_NODES = 100000
N_EDGES = 3200000
IN_CH = 256
K = 10
NCORES = 8
P = 128
NPC = N_NODES // NCORES          # 12500 nodes per core
R = 98                           # rows per core (128*98 = 12544 cells)
NPAD = P * R                     # 12544
QB = (0, 16, 47, 78, 98)         # quarter row boundaries (q0 small so gathers
                                 # start early; q3 small so the tail starts early)
NR = tuple(QB[i + 1] - QB[i] for i in range(4))       # rows per quarter
NQR = tuple(nr * P for nr in NR)                      # cells per quarter/core
WROWS = tuple(NCORES * n for n in NQR)                # window table rows
HOLE_RANK = tuple(QB[i + 1] * P - 1 for i in range(4))  # forced-zero cells
BB = (0, 28, 56, 84, 98)         # row bands for the combine/softmax/out tail
GCH = 12160                      # gather tokens per instruction (95 cols)
GCOLS = GCH // P

F32 = mybir.dt.float32
BF16 = mybir.dt.bfloat16
I16 = mybir.dt.int16

BF = ml_dtypes.bfloat16


def _raw_dma_gather(gp, out_ap, in_ap, idxs_ap, num_idxs, elem_size, elem_step,
                    single_packet=False, queue_num=0):
    """bass.dma_gather minus the 256B-elem assert (non-transpose, DRAM src)."""
    assert idxs_ap.dtype == mybir.dt.int16
    assert in_ap.space == bass.MemorySpace.DRAM
    assert idxs_ap.space == bass.MemorySpace.SBUF
    assert out_ap.space == bass.MemorySpace.SBUF
    assert in_ap.dtype == out_ap.dtype
    assert ap_utils.ap_is_contiguous(in_ap.ap[1:])
    assert ap_utils.ap_is_contiguous(out_ap.ap[1:])
    assert ap_utils.ap_is_contiguous(idxs_ap.ap[1:])
    assert in_ap.ap[-1][1] == elem_size and out_ap.ap[-1][1] == elem_size
    assert in_ap.ap[0][0] == elem_step
    stride_bytes = elem_step * mybir.dt.size(in_ap.dtype)
    stride_bytes_256 = stride_bytes // 256
    assert stride_bytes_256 * 256 == stride_bytes and 0 < stride_bytes_256 < 256
    _in_ap = gp.lower_ap_dma(in_ap, for_custom_bir_dma=True)
    _idxs_ap = gp.lower_ap(idxs_ap)
    _out_ap = gp.lower_ap(out_ap)
    return gp.add_instruction(
        mybir.InstDMAGatherAnt(
            name=gp.bass.get_next_instruction_name(),
            ins=[*_in_ap, _idxs_ap, gp.lower_val_access(gp.to_reg(num_idxs))],
            outs=[_out_ap],
            transpose=False,
            num_idxs=num_idxs,
            elem_size=elem_size,
            stride_bytes_256=stride_bytes_256,
            gen_mode=0,
            single_packet=single_packet,
            queue_num=queue_num,
            sbuf_tokens_per_rank=0,
            sbuf_free_dim_per_rank=0,
            sbuf_free_dim_pad_per_rank=0,
            sbuf_byte_offset=0,
        )
    )


# ----------------------------------------------------------------------------
# Host-side sharding / index prep (numpy, index-only)
# ----------------------------------------------------------------------------

def host_prep(edge_index):
    src = np.asarray(edge_index[0], dtype=np.int64)
    dst = np.asarray(edge_index[1], dtype=np.int64)

    deg_all = np.bincount(dst, minlength=N_NODES).astype(np.int64) + 1

    # canonical layout: per core, degree-sorted into ranks, skipping the four
    # forced-zero hole cells (one per quarter)
    holes = np.array(HOLE_RANK, dtype=np.int64)
    avail = np.setdiff1d(np.arange(NPAD, dtype=np.int64), holes)[:NPC]
    rank_of = np.empty(N_NODES, dtype=np.int64)
    for c in range(NCORES):
        lo = c * NPC
        order = np.argsort(-deg_all[lo:lo + NPC], kind="stable")
        rank_of[lo + order] = avail

    p_of = rank_of % P
    r_of = rank_of // P
    core_of = np.arange(N_NODES) // NPC
    # window of a node (as a source) = quarter of its canonical row
    qb = np.asarray(QB[1:], dtype=np.int64)
    w_of = np.searchsorted(qb, r_of, side="right")
    # window-w g-table row of node v (p-major within each core's quarter)
    nr_of = np.asarray(NR, dtype=np.int64)[w_of]
    grow_of = core_of * np.asarray(NQR, dtype=np.int64)[w_of] \
        + p_of * nr_of + (r_of - np.asarray(QB, dtype=np.int64)[w_of])

    w_e = w_of[src]

    # per-node per-window in-edge counts (incl. self-loop in own quarter)
    cw = np.bincount(w_e * N_NODES + dst,
                     minlength=4 * N_NODES).reshape(4, N_NODES)
    cw[w_of, np.arange(N_NODES)] += 1

    # one global stable sort of edges by (dst core, window); per-(c,w) groups
    # are then contiguous slices instead of 32 full-array mask passes
    ckey = (dst // NPC) * 4 + w_e
    eorder = np.argsort(ckey, kind="stable")
    cbounds = np.searchsorted(ckey[eorder], np.arange(NCORES * 4 + 1))

    # per-window destination layouts: per core sorted by cw desc
    rank_w = np.empty((4, N_NODES), np.int64)
    K_w = np.zeros((4, R), np.int64)
    for w in range(4):
        for c in range(NCORES):
            lo = c * NPC
            order = np.argsort(-cw[w, lo:lo + NPC], kind="stable")
            rank_w[w, lo + order] = np.arange(NPC)
            srt = cw[w, lo + order]
            # row max = first element of each row of 128 (desc sorted)
            rmax = np.zeros(R, np.int64)
            nrows = (NPC + P - 1) // P
            heads = srt[0:NPC:P]
            rmax[:len(heads)] = heads
            K_w[w] = np.maximum(K_w[w], rmax)
    Koff_w = np.concatenate(
        [np.zeros((4, 1), np.int64), np.cumsum(K_w, axis=1)], axis=1)[:, :-1]
    T_w = K_w.sum(axis=1) * P                      # tokens per window stream

    # combine stream: 2 gathers x 2R*128 tokens
    T_comb = 2 * R * P
    tok_off = np.zeros(7, np.int64)                # w0..w3, combA, combB, end
    for i in range(4):
        tok_off[i + 1] = tok_off[i] + T_w[i]
    tok_off[5] = tok_off[4] + T_comb
    tok_off[6] = tok_off[5] + T_comb
    TOT = int(tok_off[6])
    assert TOT % 16 == 0
    TOT16 = TOT // 16

    # instruction plan per window: greedy-pack consecutive nonzero rows <= GCOLS
    plans = []                                     # list of (w, row_list, col0)
    for w in range(4):
        r = 0
        while r < R:
            if K_w[w, r] == 0:
                r += 1
                continue
            rows = [r]
            cols = int(K_w[w, r])
            r2 = r + 1
            while r2 < R and cols + int(K_w[w, r2]) <= GCOLS:
                if int(K_w[w, r2]) > 0:
                    rows.append(r2)
                    cols += int(K_w[w, r2])
                r2 += 1
            plans.append((w, rows, int(Koff_w[w, rows[0]]), cols))
            r = r2

    # per-core token streams
    idxs = []
    degs = []
    xmaps = []
    for c in range(NCORES):
        lo, hi = c * NPC, (c + 1) * NPC
        flat = np.empty(TOT, np.int16)
        for w in range(4):
            # edges into this core from window w, plus self-loops
            grp = eorder[cbounds[c * 4 + w]:cbounds[c * 4 + w + 1]]
            e_src = src[grp]
            e_dst = dst[grp]
            sl = np.nonzero(w_of[lo:hi] == w)[0] + lo
            e_src = np.concatenate([e_src, sl])
            e_dst = np.concatenate([e_dst, sl])
            d_rank = rank_w[w, e_dst]
            # stable sort by destination cell (rank = r*128+p encodes (r,p))
            order = np.argsort(d_rank, kind="stable")
            e_src, d_rank = e_src[order], d_rank[order]
            d_p = d_rank % P
            d_r = d_rank // P
            key = d_rank
            cnts = np.bincount(key, minlength=R * P)
            k_within = np.arange(len(key)) - np.repeat(
                np.concatenate([[0], np.cumsum(cnts)])[:-1], cnts)
            t = (Koff_w[w, d_r] + k_within) * P + d_p
            seg = np.full(int(T_w[w]), c * NQR[w] + NQR[w] - 1, np.int16)
            seg[t] = grow_of[e_src].astype(np.int16)
            flat[tok_off[w]:tok_off[w + 1]] = seg

        # combine streams: canonical cell (p, r) pulls window partials
        rho = np.arange(NPAD, dtype=np.int64)
        pp, rr = rho % P, rho // P
        # node at canonical rank rho (or -1 for dummies)
        v_at = np.full(NPAD, -1, np.int64)
        v_at[rank_of[lo:hi]] = np.arange(lo, hi)
        for half in range(2):
            toks = np.empty(2 * NPAD, np.int64)
            for j in range(2):
                w = half * 2 + j
                rw = np.full(NPAD, NPAD - 1, np.int64)
                real = v_at >= 0
                rw[real] = rank_w[w, v_at[real]]
                # ctab row = j*NPAD + p_w*R + r_w  (p-major)
                crow = j * NPAD + (rw % P) * R + rw // P
                toks[(rr * 2 + j) * P + pp] = crow
            flat[tok_off[4 + half]:tok_off[5 + half]] = toks.astype(np.int16)

        wrap = flat.reshape(TOT16, 16).T
        idx32 = np.empty((32, TOT16), np.int16)
        idx32[0:16] = wrap
        idx32[16:32] = wrap
        idxs.append(np.ascontiguousarray(idx32))

        dg = np.full(NPAD, 1.0, np.float32)
        dg[rank_of[lo:hi]] = deg_all[lo:hi].astype(np.float32)
        dg[list(HOLE_RANK)] = 1e30            # dinv ~= 0 at forced-zero cells
        degs.append(np.ascontiguousarray(dg.reshape(R, P).T))

        xmaps.append(None)

    return {
        "rank_of": rank_of, "K_w": K_w, "plans": plans,
        "tok_off": tok_off, "TOT16": TOT16, "idx": idxs, "deg": degs,
    }


# ----------------------------------------------------------------------------
# Device kernel
# ----------------------------------------------------------------------------

_BUILD_CACHE = {}


def build_kernel(K_w, plans, tok_off, TOT16):
    K_w = np.asarray(K_w, dtype=np.int64)
    key = (TOT16,) + tuple(int(k) for k in K_w.ravel())
    if key in _BUILD_CACHE:
        return _BUILD_CACHE[key]
    Koff_w = np.concatenate(
        [np.zeros((4, 1), np.int64), np.cumsum(K_w, axis=1)], axis=1)[:, :-1]

    nc = bacc.Bacc("TRN2", target_bir_lowering=False, debug=False,
                   num_devices=NCORES, dynamic_dma_scratch_size=49152)

    x_in = nc.dram_tensor("x", [P, 2 * NPAD], BF16, kind="ExternalInput").ap()
    w16_in = nc.dram_tensor("w16", [P, 2 * K], BF16, kind="ExternalInput").ap()
    w_in = nc.dram_tensor("w", [IN_CH, K], F32, kind="ExternalInput").ap()
    bias_in = nc.dram_tensor("bias", [1, K], F32, kind="ExternalInput").ap()
    emb_in = nc.dram_tensor("emb", [K, IN_CH], F32, kind="ExternalInput").ap()
    deg_in = nc.dram_tensor("deg", [P, R], F32, kind="ExternalInput").ap()
    idx_in = nc.dram_tensor("idx", [32, TOT16], I16, kind="ExternalInput").ap()
    out = nc.dram_tensor("out", [P, R * IN_CH], BF16, kind="ExternalOutput").ap()

    with tile.TileContext(nc) as tc, \
         nc.allow_low_precision(reason="bf16 partial sums, 2e-2 tolerance"):
        with tc.tile_pool(name="big", bufs=1) as big, \
             tc.tile_pool(name="small", bufs=1) as small, \
             tc.tile_pool(name="pt64", bufs=1) as pt64p, \
             tc.tile_pool(name="msg", bufs=14) as msgp, \
             tc.tile_pool(name="ops", bufs=3) as opsp, \
             tc.tile_pool(name="ps0", bufs=1, space="PSUM") as ps0, \
             tc.tile_pool(name="psHW", bufs=2, space="PSUM") as psHW, \
             tc.tile_pool(name="psST", bufs=2, space="PSUM") as psST, \
             tc.tile_pool(name="psP", bufs=2, space="PSUM") as psP, \
             tc.tile_pool(name="dram", bufs=1, space="DRAM") as dram:

            # ---- resident loads (small first; x split per quarter so phase B
            # starts as soon as quarter 0's slice lands)
            w16_sb = small.tile([P, 2 * K], BF16)
            nc.sync.dma_start(w16_sb[:], w16_in[:])
            w_sb = small.tile([P, 2 * K], F32)            # f32 copy for cW
            nc.sync.dma_start(w_sb[:, 0:K], w_in[0:P, :])
            nc.sync.dma_start(w_sb[:, K:2 * K], w_in[P:2 * P, :])
            emb_sb = small.tile([K, IN_CH], F32)
            nc.sync.dma_start(emb_sb[:], emb_in[:])
            deg_sb = small.tile([P, R], F32)
            nc.sync.dma_start(deg_sb[:], deg_in[:])
            bias_sb = small.tile([1, K], F32)
            nc.sync.dma_start(bias_sb[:], bias_in[:])
            idx_sb = big.tile([32, TOT16], I16)           # resident idx stream
            nc.sync.dma_start(idx_sb[:], idx_in[:])
            x_sb = big.tile([P, 2 * NPAD], BF16)          # 50KB/part
            for q in range(4):
                lo, hi = QB[q] * P, QB[q + 1] * P
                for h in range(2):
                    nc.sync.dma_start(x_sb[:, h * NPAD + lo:h * NPAD + hi],
                                      x_in[:, h * NPAD + lo:h * NPAD + hi])

            ident = small.tile([P, P], F32)
            make_identity(nc, ident[:])

            ones_row = small.tile([1, P], F32)
            nc.vector.memset(ones_row[:], 1.0)
            ones_row16 = small.tile([1, P], BF16)
            nc.vector.memset(ones_row16[:], 1.0)
            ones_col10 = small.tile([K, 1], F32)
            nc.vector.memset(ones_col10[:], 1.0)

            # ---- dinv = 1/sqrt(deg)  (deg=inf at hole cells -> dinv=0)
            dinv_sb = small.tile([P, R], F32)
            nc.scalar.activation(dinv_sb[:], deg_sb[:],
                                 mybir.ActivationFunctionType.Sqrt)
            nc.vector.reciprocal(dinv_sb[:], dinv_sb[:])

            # ---- cW = (sum_k emb[k]) @ W  as [1, 10]  (f32, then bf16)
            csumT_ps = ps0.tile([P, 2], F32, space="PSUM", tag="t0")
            for h in range(2):
                nc.tensor.matmul(csumT_ps[:, h:h + 1],
                                 lhsT=emb_sb[:, h * P:(h + 1) * P],
                                 rhs=ones_col10[:], start=True, stop=True)
            csumT = small.tile([P, 2], F32)
            nc.vector.tensor_copy(csumT[:], csumT_ps[:])
            cw_ps = ps0.tile([1, K], F32, space="PSUM", tag="t0")
            for h in range(2):
                nc.tensor.matmul(cw_ps[:], lhsT=csumT[:, h:h + 1],
                                 rhs=w_sb[:, h * K:(h + 1) * K],
                                 start=(h == 0), stop=(h == 1))
            cw16_sb = small.tile([1, K], BF16)
            nc.vector.tensor_copy(cw16_sb[:], cw_ps[:])

            # bias broadcast to all partitions: [128, 10]
            biasb_ps = ps0.tile([P, K], F32, space="PSUM", tag="t0")
            nc.tensor.matmul(biasb_ps[:], lhsT=ones_row[:], rhs=bias_sb[:],
                             start=True, stop=True)
            biasb = small.tile([P, K], F32)
            nc.vector.tensor_copy(biasb[:], biasb_ps[:])

            # emb in bf16 for the final matmul
            emb16_sb = small.tile([K, IN_CH], BF16)
            nc.vector.tensor_copy(emb16_sb[:], emb_sb[:])

            # ---- phase B: g = dinv * (x @ W + cW), published per quarter
            # (bf16 tables: halves collective bytes; 0.4% rounding is far
            # inside the 2e-2 tolerance)
            g_sb = big.tile([P, R * K], BF16)
            tables = []
            for q in range(4):
                for r in range(QB[q], QB[q + 1]):
                    hw_ps = psHW.tile([P, K], F32, space="PSUM", tag="hw")
                    nc.tensor.matmul(hw_ps[:],
                                     lhsT=x_sb[:, r * P:(r + 1) * P],
                                     rhs=w16_sb[:, 0:K], start=True, stop=False)
                    nc.tensor.matmul(hw_ps[:],
                                     lhsT=x_sb[:, NPAD + r * P:NPAD + (r + 1) * P],
                                     rhs=w16_sb[:, K:2 * K], start=False, stop=False)
                    nc.tensor.matmul(hw_ps[:], lhsT=ones_row16[:],
                                     rhs=cw16_sb[:], start=False, stop=True)
                    nc.vector.tensor_scalar_mul(
                        g_sb[:, r * K:(r + 1) * K], hw_ps[:], dinv_sb[:, r:r + 1])
                # bounce quarter q compact (p-major rows) + allgather + expand
                qtab = dram.tile([NQR[q], K], BF16, tag=f"qtab{q}")
                nc.sync.dma_start(
                    qtab[:].rearrange("(p r) k -> p (r k)", p=P),
                    g_sb[:, QB[q] * K:QB[q + 1] * K])
                gall = dram.tile([WROWS[q], K], BF16, tag=f"gall{q}")
                nc.gpsimd.collective_compute(
                    "AllGather", mybir.AluOpType.bypass,
                    replica_groups=[list(range(NCORES))],
                    ins=[qtab[:].opt()],
                    outs=[gall[:].opt()],
                )
                table = dram.tile([WROWS[q], 128], BF16, tag=f"table{q}")
                # expand on the (otherwise idle) ACT queue so it never blocks
                # the SP queue that feeds gather idx loads
                nc.scalar.dma_start(table[:, 0:K], gall[:])
                tables.append(table)

            # ---- windowed gathers + per-row partial reduces (window layouts)
            ctab = dram.tile([4 * NPAD, 128], BF16, tag="ctab")
            cg0 = big.tile([P, 2 * R * K], BF16, tag="cg0")
            cg1 = big.tile([P, 2 * R * K], BF16, tag="cg1")
            cg = [cg0, cg1]

            def emit_combine(half):
                # one gather per row band; tokens (r,j)-column-major
                for b in range(4):
                    c0, c1 = 2 * BB[b], 2 * BB[b + 1]
                    n = (c1 - c0) * P
                    tok0 = int(tok_off[4 + half]) + c0 * P
                    _raw_dma_gather(
                        nc.gpsimd,
                        cg[half][:, c0 * K:c1 * K].rearrange(
                            "p (c j) -> p c j", j=K),
                        ctab[2 * half * NPAD:2 * (half + 1) * NPAD, 0:K],
                        idx_sb[:, tok0 // 16:(tok0 + n) // 16], n, K, 128,
                        single_packet=False)

            parts = []
            for w in range(4):
                pt = pt64p.tile([P, R * K], BF16, tag=f"pt{w}")
                nc.vector.memset(pt[:], 0.0)
                parts.append(pt)

            def publish(w):
                # publish finished window partial to its ctab block
                # (the DMA itself does the 40B->256B stride expansion)
                nc.scalar.dma_start(
                    ctab[w * NPAD:(w + 1) * NPAD, 0:K].rearrange(
                        "(p r) k -> p r k", p=P),
                    parts[w][:].rearrange("p (r k) -> p r k", k=K))

            cur_w = -1
            for (w, rows, koff0, cols) in plans:
                if w != cur_w:
                    if cur_w >= 0:
                        publish(cur_w)
                        if cur_w == 2:
                            # A-half (windows 0,1) re-alignment gather; emitted
                            # here so it never queues ahead of ready w2 work
                            emit_combine(0)
                    part = parts[w]
                    cur_w = w
                # gather instructions for this row-pack (<= GCOLS columns)
                c0 = 0
                while c0 < cols:
                    cc = min(cols - c0, GCOLS)
                    n = cc * P
                    tok0 = int(tok_off[w]) + (koff0 + c0) * P
                    msg = msgp.tile([P, GCOLS * K], BF16, tag="msgbuf")
                    _raw_dma_gather(
                        nc.gpsimd,
                        msg[:, 0:cc * K].rearrange("p (c j) -> p c j", j=K),
                        tables[w][:, 0:K],
                        idx_sb[:, tok0 // 16:(tok0 + n) // 16], n, K, 128,
                        single_packet=False)
                    # per-row reduces out of the packed tile (bf16 partials
                    # are fine: <=31-term sums vs a 2e-2 output tolerance)
                    off = -koff0 - c0
                    for rr in rows:
                        kk = int(K_w[w, rr])
                        lo = int(Koff_w[w, rr]) + off
                        hi = lo + kk
                        lo2, hi2 = max(lo, 0), min(hi, cc)
                        if hi2 <= lo2:
                            continue
                        if kk == hi2 - lo2:
                            nc.vector.tensor_reduce(
                                out=part[:, rr * K:(rr + 1) * K],
                                in_=msg[:, lo2 * K:hi2 * K].rearrange(
                                    "p (c j) -> p j c", j=K),
                                axis=mybir.AxisListType.X,
                                op=mybir.AluOpType.add)
                        else:
                            # row split across chunks: accumulate
                            tmp = opsp.tile([P, K], F32, tag="rtmp")
                            nc.vector.tensor_reduce(
                                out=tmp[:],
                                in_=msg[:, lo2 * K:hi2 * K].rearrange(
                                    "p (c j) -> p j c", j=K),
                                axis=mybir.AxisListType.X,
                                op=mybir.AluOpType.add)
                            nc.vector.tensor_add(
                                out=part[:, rr * K:(rr + 1) * K],
                                in0=part[:, rr * K:(rr + 1) * K],
                                in1=tmp[:])
                    c0 += cc
            publish(cur_w)
            emit_combine(1)

            # ---- per-band tail: logits -> softmax -> out^T = emb^T s^T + x^T
            lg = big.tile([P, R * K], F32)
            tmpA = big.tile([P, 28 * K], F32, tag="tmpA")
            den = small.tile([P, R], F32)
            for b in range(4):
                r0b, r1b = BB[b], BB[b + 1]
                nb = r1b - r0b
                lg3 = lg[:, r0b * K:r1b * K].rearrange("p (r j) -> p r j", j=K)
                a4 = cg[0][:, 2 * r0b * K:2 * r1b * K].rearrange(
                    "p (r i j) -> p r i j", i=2, j=K)
                b4 = cg[1][:, 2 * r0b * K:2 * r1b * K].rearrange(
                    "p (r i j) -> p r i j", i=2, j=K)
                tA = tmpA[:, 0:nb * K].rearrange("p (r j) -> p r j", j=K)
                nc.vector.tensor_tensor(out=tA, in0=a4[:, :, 0, :],
                                        in1=a4[:, :, 1, :],
                                        op=mybir.AluOpType.add)
                nc.vector.tensor_tensor(out=lg3, in0=b4[:, :, 0, :],
                                        in1=b4[:, :, 1, :],
                                        op=mybir.AluOpType.add)
                nc.vector.tensor_tensor(out=lg3, in0=lg3, in1=tA,
                                        op=mybir.AluOpType.add)
                nc.vector.tensor_tensor(
                    out=lg3, in0=lg3,
                    in1=dinv_sb[:, r0b:r1b].unsqueeze(2).to_broadcast(
                        [P, nb, K]),
                    op=mybir.AluOpType.mult)
                nc.vector.tensor_tensor(
                    out=lg3, in0=lg3,
                    in1=biasb[:].unsqueeze(1).to_broadcast([P, nb, K]),
                    op=mybir.AluOpType.add)
                nc.scalar.activation(lg[:, r0b * K:r1b * K],
                                     lg[:, r0b * K:r1b * K],
                                     mybir.ActivationFunctionType.Exp)
                nc.vector.tensor_reduce(out=den[:, r0b:r1b], in_=lg3,
                                        axis=mybir.AxisListType.X,
                                        op=mybir.AluOpType.add)
                nc.vector.reciprocal(den[:, r0b:r1b], den[:, r0b:r1b])
                nc.vector.tensor_tensor(
                    out=lg3, in0=lg3,
                    in1=den[:, r0b:r1b].unsqueeze(2).to_broadcast([P, nb, K]),
                    op=mybir.AluOpType.mult)

                for r0 in range(r0b, r1b, 4):
                    nr = min(4, r1b - r0)
                    st_ps = psST.tile([K, 4 * P], F32, space="PSUM", tag="stps")
                    for j in range(nr):
                        nc.tensor.transpose(
                            st_ps[:, j * P:(j + 1) * P],
                            lg[:, (r0 + j) * K:(r0 + j + 1) * K], ident[:])
                    st = opsp.tile([K, 4 * P], BF16, tag="st")
                    nc.vector.tensor_copy(st[:, 0:nr * P], st_ps[:, 0:nr * P])
                    out_t = opsp.tile([P, 2 * 4 * P], BF16, tag="outt")
                    for h in range(2):
                        p_ps = psP.tile([P, 4 * P], F32, space="PSUM", tag="pps")
                        nc.tensor.matmul(p_ps[:, 0:nr * P],
                                         lhsT=emb16_sb[:, h * P:(h + 1) * P],
                                         rhs=st[:, 0:nr * P],
                                         start=True, stop=True)
                        nc.vector.tensor_add(
                            out=out_t[:, h * nr * P:(h + 1) * nr * P],
                            in0=p_ps[:, 0:nr * P],
                            in1=x_sb[:, h * NPAD + r0 * P:
                                     h * NPAD + (r0 + nr) * P])
                    nc.sync.dma_start(
                        out[:, r0 * 2 * P:(r0 + nr) * 2 * P],
                        out_t[:, 0:nr * 2 * P])

    nc.compile()
    _BUILD_CACHE[key] = nc
    return nc


# ----------------------------------------------------------------------------
# Entry point
# ----------------------------------------------------------------------------

def kernel(x, edge_index, batch, W, bias, cluster_emb):
    _t = [time.time()]
    def _tick(label):
        _t.append(time.time())
        if os.environ.get("KV2_TIMING"):
            print(f"[kv2] {label}: {_t[-1]-_t[-2]:.2f}s", flush=True)
    x = np.asarray(x, dtype=np.float32)
    W = np.asarray(W, dtype=np.float32)
    bias = np.asarray(bias, dtype=np.float32).reshape(1, K)
    cluster_emb = np.asarray(cluster_emb, dtype=np.float32)

    plan = host_prep(edge_index)
    _tick("host_prep")
    nc = build_kernel(plan["K_w"], plan["plans"], plan["tok_off"], plan["TOT16"])
    _tick("build_kernel")

    w16 = np.ascontiguousarray(
        W.reshape(2, P, K).transpose(1, 0, 2).reshape(P, 2 * K).astype(BF))

    in_maps = []
    for c in range(NCORES):
        lo = c * NPC
        rank = plan["rank_of"][lo:lo + NPC]
        # x channel-major bf16: x_in[p, h*NPAD + rank] = x[v, h*128+p]
        xt = np.zeros((NPAD, IN_CH), dtype=BF)
        xt[rank] = x[lo:lo + NPC].astype(BF)
        xp = xt.reshape(NPAD, 2, P).transpose(2, 1, 0).reshape(P, 2 * NPAD)
        in_maps.append({
            "x": np.ascontiguousarray(xp),
            "w16": w16,
            "w": W,
            "bias": bias,
            "emb": cluster_emb,
            "deg": plan["deg"][c],
            "idx": plan["idx"][c],
        })

    _tick("in_maps")
    res = run_bass_kernel_spmd(nc, in_maps, core_ids=list(range(NCORES)))
    _tick("run_bass_kernel_spmd")

    out = np.empty((N_NODES, IN_CH), dtype=np.float32)
    for c in range(NCORES):
        o = np.asarray(res.results[c]["out"])          # [128, R*256] bf16
        # per 4-row group: cols = h*nr*128 + j*128 + n; node=(r0+j)*128+n,
        # ch = h*128+p
        on = np.empty((NPAD, IN_CH), dtype=np.float32)
        col = 0
        for g in range((R + 3) // 4):
            r0 = g * 4
            nr = min(4, R - r0)
            blk = o[:, col:col + nr * 2 * P].reshape(P, 2, nr, P)
            on[r0 * P:(r0 + nr) * P] = (
                blk.transpose(2, 3, 1, 0).reshape(nr * P, IN_CH)
                .astype(np.float32))
            col += nr * 2 * P
        lo = c * NPC
        out[lo:lo + NPC] = on[plan["rank_of"][lo:lo + NPC]]
    _tick("decode")
    return out
